# revision 1
# baseline (speedup 1.0000x reference)
"""Trainium2 Bass kernel for the gnn_message_passing problem.

Math refactor: the reference computes
    kernel[z,i,j] = einsum('zk,kij->zij', Rk*Yk, cg) * nc0[i,j]
with Rk = R @ rf_mix.T (rank 6) and Yk = Y.T @ ylm_mix.T (rank 9).
Since Rk*Yk has rank<=54 over k, fold the K=1024 contraction into a
precomputed M[p*9+l, ij] = sum_k rf[k,p]*ylm_s[k,l]*cg[k,ij] * nc0[ij]
(computed on device from the cg/rf/ylm/norm inputs), and per point only
contract B[z, pl] = R[z,p]*Y'[z,l] against M - a k=54 matmul. This cuts
compute ~20x and makes the kernel memory-bound (410 MB output).

Distribution: data-parallel over z across 8 NeuronCores; constants
replicated. Full inputs in, full output out.

Precision: the fast fp32r matmul path rounds inputs to an 11-bit
mantissa, so the main contraction uses a hi/lo split:
    out = [Bh;Bl] @ [Mh;Mh]  (k=108, accumulating)  +  Bh @ Ml  (k=54)
which drops only the Bl@Ml term (~2^-24 relative) - fp32-class accuracy
at 1 cycle/row. Everything feeding B (monomials, radial MLP) runs in
exact fp32 (PE fp32 mode, DVE reciprocal, ACT sqrt + one Newton step).
"""

import numpy as np

import concourse.bass as bass
import concourse.tile as tile
from concourse import bacc, mybir
from concourse.bass_utils import run_bass_kernel_spmd

F32 = mybir.dt.float32
F32R = mybir.dt.float32r
ALU = mybir.AluOpType
ACTF = mybir.ActivationFunctionType

# Problem shape (hardcoded per contract)
Z, KDIM, DO, DI, NPATH, H = 100000, 1024, 32, 32, 6, 128
IJ = DO * DI                      # 1024
NCORES = 8
ZC = Z // NCORES                  # 12500 points per core
T = 100                           # point tiles of 128 -> ZC padded to 12800
ZC_PAD = 128 * T
TB = 4                            # tiles per group
NG = T // TB                      # 25 groups
NCH = 10                          # channels: radii, ones, 8 scaled monomials
NKT = KDIM // 128                 # 8 k-tiles for the M build

# Real spherical harmonic constants (l=0,1,2), folded into ylm host-side
C0 = 0.28209479177387814
C1 = 0.4886025119029199
C2A = 1.0925484305920792
C2B = 0.31539156525252005
C2C = 0.5462742152960396
YLM_SCALE = np.array([C0, C1, C1, C1, C2A, C2A, C2B, C2A, C2C], dtype=np.float64)

_CACHE = {}


def _build_program():
    nc = bacc.Bacc("TRN2", target_bir_lowering=False, debug=False,
                   num_devices=NCORES)

    # ---- per-core DRAM I/O ----
    rpl = nc.dram_tensor("rpl", [128, 3 * T], F32, kind="ExternalInput").ap()
    w1e4 = nc.dram_tensor("w1e4", [NCH * TB, TB * 128], F32, kind="ExternalInput").ap()
    ey4 = nc.dram_tensor("ey4", [NCH * TB, TB * 54], F32, kind="ExternalInput").ap()
    w2e = nc.dram_tensor("w2e", [H, 54], F32, kind="ExternalInput").ap()
    b1c = nc.dram_tensor("b1c", [H, 1], F32, kind="ExternalInput").ap()
    b2r = nc.dram_tensor("b2r", [54, 1], F32, kind="ExternalInput").ap()
    cgd = nc.dram_tensor("cgd", [KDIM, IJ], F32, kind="ExternalInput").ap()
    rft = nc.dram_tensor("rft", [128, NKT * NPATH], F32, kind="ExternalInput").ap()
    ylt = nc.dram_tensor("ylt", [128, NKT * 9], F32, kind="ExternalInput").ap()
    ncv = nc.dram_tensor("ncv", [1, IJ], F32, kind="ExternalInput").ap()
    identd = nc.dram_tensor("identd", [128, 128], F32, kind="ExternalInput").ap()
    out = nc.dram_tensor("out", [ZC, IJ], F32, kind="ExternalOutput").ap()

    with tile.TileContext(nc) as tc:
        with tc.tile_pool(name="const", bufs=1) as cpool, \
             tc.tile_pool(name="mbuf", bufs=1) as mpool:
            # ---- resident constants ----
            w1e_sb = cpool.tile([NCH * TB, TB * 128], F32)
            nc.sync.dma_start(w1e_sb[:], w1e4[:])
            ey4_sb = cpool.tile([NCH * TB, TB * 54], F32)
            nc.sync.dma_start(ey4_sb[:], ey4[:])
            w2e_sb = cpool.tile([H, 54], F32)
            nc.sync.dma_start(w2e_sb[:], w2e[:])
            b1_sb = cpool.tile([H, 1], F32)
            nc.sync.dma_start(b1_sb[:], b1c[:])
            b2_sb = cpool.tile([54, 1], F32)
            nc.sync.dma_start(b2_sb[:], b2r[:])
            id_sb = cpool.tile([128, 128], F32)
            nc.sync.dma_start(id_sb[:], identd[:])
            ncv_sb = cpool.tile([1, IJ], F32)
            nc.sync.dma_start(ncv_sb[:], ncv[:])
            ones54 = cpool.tile([1, 54], F32)
            nc.vector.memset(ones54[:], 1.0)

            # M tensors: rows 0-53 = Mh, 54-63 = zeros (partition-alignment
            # filler; engine writes must start at 0/32/64/96), 64-117 = Mh.
            mstk = cpool.tile([118, IJ], F32R)
            ml_sb = cpool.tile([54, IJ], F32R)
            # B hi/lo stack: 3 manually rotated slots along the free dim
            bstk_all = cpool.tile([118, 6 * 128], F32R)
            nc.vector.memset(mstk[32:64, :].bitcast(F32), 0.0)
            nc.vector.memset(bstk_all[32:64, :].bitcast(F32), 0.0)

            # =========================================================
            # Phase 1: build M[pl, ij] from cg, rf, ylm, norm_coef
            # =========================================================
            with tc.tile_pool(name="mpsum", bufs=1, space="PSUM") as mps_pool:
                cg_sb = mpool.tile([128, NKT * IJ], F32)
                cg_r = cgd.rearrange("(kt p) ij -> p kt ij", p=128)
                nc.sync.dma_start(
                    cg_sb[:].rearrange("p (kt ij) -> p kt ij", kt=NKT), cg_r)
                rf_sb = mpool.tile([128, NKT * NPATH], F32)
                nc.sync.dma_start(rf_sb[:], rft[:])
                yl_sb = mpool.tile([128, NKT * 9], F32)
                nc.sync.dma_start(yl_sb[:], ylt[:])

                # W[k, pl] = rf[k,p] * ylm_s[k,l]
                w_sb = mpool.tile([128, NKT * 54], F32)
                for kt in range(NKT):
                    for p in range(NPATH):
                        nc.vector.tensor_scalar(
                            w_sb[:, kt * 54 + p * 9: kt * 54 + p * 9 + 9],
                            yl_sb[:, kt * 9: kt * 9 + 9],
                            rf_sb[:, kt * NPATH + p: kt * NPATH + p + 1],
                            None, ALU.mult)

                m_ps = mps_pool.tile([54, IJ], F32)
                for half in range(2):
                    for kt in range(NKT):
                        nc.tensor.matmul(
                            m_ps[:, half * 512:(half + 1) * 512],
                            w_sb[:, kt * 54:(kt + 1) * 54],
                            cg_sb[:, kt * IJ + half * 512: kt * IJ + half * 512 + 512],
                            start=(kt == 0), stop=(kt == NKT - 1))

                # broadcast norm_coef[...,0] across the 54 partitions
                ncr_ps = mps_pool.tile([54, IJ], F32)
                for half in range(2):
                    nc.tensor.matmul(
                        ncr_ps[:, half * 512:(half + 1) * 512],
                        ones54[:],
                        ncv_sb[:, half * 512:(half + 1) * 512],
                        start=True, stop=True)
                ncr_sb = mpool.tile([54, IJ], F32)
                nc.scalar.copy(ncr_sb[:], ncr_ps[:])

                mf_sb = mpool.tile([54, IJ], F32)
                nc.vector.tensor_tensor(mf_sb[:], m_ps[:], ncr_sb[:], ALU.mult)
                # hi/lo split (fp32r rounding happens on write)
                nc.vector.tensor_copy(mstk[0:54, :], mf_sb[:])
                nc.scalar.copy(mstk[64:118, :], mf_sb[:])
                nc.vector.tensor_tensor(ml_sb[:], mf_sb[:],
                                        mstk[0:54, :].bitcast(F32), ALU.subtract)

            # =========================================================
            # Phase 2: per-point planes [128, T]: radii, ones, monomials
            # =========================================================
            rpl_sb = cpool.tile([128, 3 * T], F32)
            nc.sync.dma_start(rpl_sb[:], rpl[:])
            x = rpl_sb[:, 0:T]
            y = rpl_sb[:, T:2 * T]
            z = rpl_sb[:, 2 * T:3 * T]

            chan = cpool.tile([128, NCH * T], F32)
            aux = cpool.tile([128, 10 * T], F32)

            def ax(i):
                return aux[:, i * T:(i + 1) * T]

            xx, yy, zz, s1, r2, mask, inv2, va, vb, t8 = (ax(i) for i in range(10))
            nc.vector.tensor_tensor(xx, x, x, ALU.mult)
            nc.vector.tensor_tensor(yy, y, y, ALU.mult)
            nc.vector.tensor_tensor(zz, z, z, ALU.mult)
            nc.vector.tensor_tensor(s1, xx, yy, ALU.add)
            nc.vector.tensor_tensor(r2, s1, zz, ALU.add)
            # guard r2 == 0 exactly like the reference's safe_r2
            nc.vector.tensor_scalar(mask, r2, 0.0, None, ALU.is_equal)
            nc.vector.tensor_tensor(mask, r2, mask, ALU.add)     # safe_r2
            nc.vector.reciprocal(inv2, mask)                     # 1/safe_r2 (accurate)
            nc.scalar.sqrt(va, inv2)                             # rsqrt seed ~7e-6
            # one Newton step: v = v*(1.5 - 0.5*safe_r2*v^2)
            nc.vector.tensor_tensor(vb, va, va, ALU.mult)
            nc.vector.tensor_tensor(vb, vb, mask, ALU.mult)
            nc.vector.tensor_scalar(vb, vb, -0.5, 1.5, ALU.mult, ALU.add)
            nc.vector.tensor_tensor(va, va, vb, ALU.mult)        # inv_r

            # chan is stored t-major interleaved (col = t*NCH + c) so each
            # group's transpose input is one contiguous 40-col slice
            chan_v = chan[:].rearrange("p (t c) -> p c t", c=NCH)
            ch = [chan_v[:, i, :] for i in range(NCH)]
            nc.vector.tensor_tensor(ch[0], r2, va, ALU.mult)     # radii
            nc.vector.tensor_scalar(ch[1], r2, 0.0, 1.0, ALU.mult, ALU.add)  # ones
            nc.vector.tensor_tensor(ch[2], y, va, ALU.mult)      # l=1
            nc.vector.tensor_tensor(ch[3], z, va, ALU.mult)      # l=2
            nc.vector.tensor_tensor(ch[4], x, va, ALU.mult)      # l=3
            nc.vector.tensor_tensor(vb, x, y, ALU.mult)
            nc.vector.tensor_tensor(ch[5], vb, inv2, ALU.mult)   # l=4: xy/r2
            nc.vector.tensor_tensor(vb, y, z, ALU.mult)
            nc.vector.tensor_tensor(ch[6], vb, inv2, ALU.mult)   # l=5: yz/r2
            nc.vector.scalar_tensor_tensor(vb, zz, 3.0, r2, ALU.mult, ALU.subtract)
            nc.vector.tensor_tensor(ch[7], vb, inv2, ALU.mult)   # l=6: (3zz-r2)/r2
            nc.vector.tensor_tensor(vb, x, z, ALU.mult)
            nc.vector.tensor_tensor(ch[8], vb, inv2, ALU.mult)   # l=7: xz/r2
            nc.vector.tensor_tensor(t8, xx, yy, ALU.subtract)
            nc.vector.tensor_tensor(ch[9], t8, inv2, ALU.mult)   # l=8: (xx-yy)/r2

            # =========================================================
            # Phase 3: main loop over 25 groups of 4 point-tiles
            # =========================================================
            with tc.tile_pool(name="tps", bufs=2, space="PSUM") as tps_pool, \
                 tc.tile_pool(name="hps", bufs=1, space="PSUM") as hps_pool, \
                 tc.tile_pool(name="rps", bufs=1, space="PSUM") as rps_pool, \
                 tc.tile_pool(name="yps", bufs=1, space="PSUM") as yps_pool, \
                 tc.tile_pool(name="kps", bufs=3, space="PSUM") as kps_pool, \
                 tc.tile_pool(name="work", bufs=2) as wpool, \
                 tc.tile_pool(name="bwork", bufs=4) as bpool, \
                 tc.tile_pool(name="kout", bufs=3) as kpool:
                for g in range(NG):
                    t0 = TB * g
                    # transpose 4 tiles x 10 channels -> [40, 128]
                    t_ps = tps_pool.tile([NCH * TB, 128], F32)
                    nc.tensor.transpose(
                        t_ps[:], chan[:, NCH * t0:NCH * t0 + NCH * TB], id_sb[:])
                    t_sb = wpool.tile([NCH * TB, 128], F32, tag="t_sb")
                    nc.scalar.copy(t_sb[:], t_ps[:])

                    # radial MLP hidden layer for the whole group
                    h_ps = hps_pool.tile([128, TB * 128], F32)
                    for dt in range(TB):
                        nc.tensor.matmul(
                            h_ps[:, dt * 128:(dt + 1) * 128],
                            w1e_sb[:, dt * 128:(dt + 1) * 128],
                            t_sb[:], start=True, stop=True)
                    h_sb = wpool.tile([128, TB * 128], F32, tag="h_sb")
                    nc.scalar.activation(h_sb[:], h_ps[:], ACTF.Relu, bias=b1_sb[:])

                    r_ps = rps_pool.tile([54, TB * 128], F32)
                    nc.tensor.matmul(r_ps[:], w2e_sb[:], h_sb[:],
                                     start=True, stop=True)
                    y_ps = yps_pool.tile([54, TB * 128], F32)
                    for dt in range(TB):
                        nc.tensor.matmul(
                            y_ps[:, dt * 128:(dt + 1) * 128],
                            ey4_sb[:, dt * 54:(dt + 1) * 54],
                            t_sb[:], start=True, stop=True)

                    # B = (R + b2) * Y', split hi/lo for the fp32r contraction
                    b1g = wpool.tile([54, TB * 128], F32, tag="b1g")
                    nc.vector.tensor_scalar(b1g[:], r_ps[:], b2_sb[:],
                                            None, ALU.add)

                    k_sb = kpool.tile([128, TB * IJ], F32, tag="k_sb")
                    for dt in range(TB):
                        bf = bpool.tile([54, 128], F32, tag="bf")
                        nc.vector.tensor_tensor(
                            bf[:], b1g[:, dt * 128:(dt + 1) * 128],
                            y_ps[:, dt * 128:(dt + 1) * 128], ALU.mult)
                        slot = (g * TB + dt) % 6
                        bstk = bstk_all[:, slot * 128:(slot + 1) * 128]
                        nc.vector.tensor_copy(bstk[0:54, :], bf[:])
                        nc.vector.tensor_tensor(
                            bstk[64:118, :], bf[:],
                            bstk[0:54, :].bitcast(F32), ALU.subtract)

                        for half in range(2):
                            k_ps = kps_pool.tile([128, 512], F32, tag="k_ps")
                            nc.tensor.matmul(
                                k_ps[:], bstk[:],
                                mstk[:, half * 512:(half + 1) * 512],
                                start=True, stop=False)
                            nc.tensor.matmul(
                                k_ps[:], bstk[0:54, :],
                                ml_sb[:, half * 512:(half + 1) * 512],
                                start=False, stop=True)
                            dest = k_sb[:, dt * IJ + half * 512:
                                        dt * IJ + (half + 1) * 512]
                            if (dt * 2 + half) % 4 == 3:
                                nc.vector.tensor_copy(dest, k_ps[:])
                            else:
                                nc.scalar.copy(dest, k_ps[:])

                    # store: group covers z rows [512g, 512g+512)
                    z0 = 512 * g
                    if z0 + 512 <= ZC:
                        for hfg in range(2):
                            og = out[z0 + hfg * 256:z0 + hfg * 256 + 256, :].rearrange(
                                "(dt pg) ij -> pg dt ij", dt=2)
                            nc.sync.dma_start(
                                og, k_sb[:, hfg * 2 * IJ:(hfg + 1) * 2 * IJ].rearrange(
                                    "pg (dt ij) -> pg dt ij", dt=2))
                    else:
                        # last group: tiles beyond ZC are padding
                        for dt in range(TB):
                            zt = z0 + dt * 128
                            if zt >= ZC:
                                break
                            rows = min(128, ZC - zt)
                            nc.sync.dma_start(
                                out[zt:zt + rows, :],
                                k_sb[0:rows, dt * IJ:(dt + 1) * IJ])
    nc.compile()
    return nc


def _get_program():
    if "nc" not in _CACHE:
        _CACHE["nc"] = _build_program()
    return _CACHE["nc"]


def _host_prep(r, W1, b1, W2, b2, cg, ylm_mix, rf_mix, norm_coef):
    r = np.asarray(r, dtype=np.float32)
    W1 = np.asarray(W1, dtype=np.float32)
    b1 = np.asarray(b1, dtype=np.float32)
    W2 = np.asarray(W2, dtype=np.float32)
    b2 = np.asarray(b2, dtype=np.float32)
    cg = np.asarray(cg, dtype=np.float32)
    ylm_mix = np.asarray(ylm_mix, dtype=np.float32)
    rf_mix = np.asarray(rf_mix, dtype=np.float32)
    norm_coef = np.asarray(norm_coef, dtype=np.float32)

    w1e4 = np.zeros((NCH * TB, TB * 128), dtype=np.float32)
    ey4 = np.zeros((NCH * TB, TB * 54), dtype=np.float32)
    for dt in range(TB):
        w1e4[NCH * dt, dt * 128:(dt + 1) * 128] = W1[0]
        for l in range(9):
            for p in range(NPATH):
                ey4[NCH * dt + 1 + l, dt * 54 + p * 9 + l] = 1.0

    ylm_s = (ylm_mix.astype(np.float64) * YLM_SCALE[None, :]).astype(np.float32)
    shared = {
        "w1e4": w1e4,
        "ey4": ey4,
        "w2e": np.ascontiguousarray(np.repeat(W2, 9, axis=1)),
        "b1c": np.ascontiguousarray(b1.reshape(H, 1)),
        "b2r": np.ascontiguousarray(np.repeat(b2, 9).reshape(54, 1)),
        "cgd": np.ascontiguousarray(cg.reshape(KDIM, IJ)),
        "rft": np.ascontiguousarray(
            rf_mix.reshape(NKT, 128, NPATH).transpose(1, 0, 2).reshape(128, NKT * NPATH)),
        "ylt": np.ascontiguousarray(
            ylm_s.reshape(NKT, 128, 9).transpose(1, 0, 2).reshape(128, NKT * 9)),
        "ncv": np.ascontiguousarray(norm_coef[:, :, 0].reshape(1, IJ)),
        "identd": np.eye(128, dtype=np.float32),
    }

    in_maps = []
    for c in range(NCORES):
        rs = r[c * ZC:(c + 1) * ZC]
        rp = np.empty((ZC_PAD, 3), dtype=np.float32)
        rp[:ZC] = rs
        rp[ZC:] = np.array([1.0, 0.0, 0.0], dtype=np.float32)
        rpl = rp.reshape(T, 128, 3).transpose(1, 2, 0).reshape(128, 3 * T)
        m = dict(shared)
        m["rpl"] = np.ascontiguousarray(rpl)
        in_maps.append(m)
    return in_maps


def _run_device(in_maps, trace=False, **kw):
    nc = _get_program()
    return run_bass_kernel_spmd(nc, in_maps, core_ids=list(range(NCORES)),
                                trace=trace, **kw)


def kernel(r, W1, b1, W2, b2, cg, ylm_mix, rf_mix, norm_coef):
    r = np.asarray(r, dtype=np.float32)
    norm_coef_f = np.asarray(norm_coef, dtype=np.float32)
    in_maps = _host_prep(r, W1, b1, W2, b2, cg, ylm_mix, rf_mix, norm_coef_f)
    res = _run_device(in_maps)
    out = np.concatenate([res.results[c]["out"] for c in range(NCORES)], axis=0)

    # points with exactly zero radius use norm_coef[..., 1] instead of [..., 0]
    x, y, z = r[:, 0], r[:, 1], r[:, 2]
    r2 = (x * x + y * y) + z * z
    zero = r2 == np.float32(0.0)
    if np.any(zero):
        scale = (norm_coef_f[:, :, 1].astype(np.float64)
                 / norm_coef_f[:, :, 0].astype(np.float64)).reshape(1, IJ)
        out[zero] = (out[zero].astype(np.float64) * scale).astype(np.float32)

    return out.reshape(Z, DO, DI)



# revision 13
# speedup vs baseline: 1.6849x; 1.6849x over previous
"""Trainium2 Bass kernel for the gnn_message_passing problem.

Math refactor: the reference computes
    kernel[z,i,j] = einsum('zk,kij->zij', Rk*Yk, cg) * nc0[i,j]
with Rk = R @ rf_mix.T (rank 6 over paths) and Yk = Y.T @ ylm_mix.T
(rank 9 over spherical harmonics).  Rk*Yk therefore lives in a rank<=54
subspace of k, so the whole K=1024 contraction folds into a constant
    M[p*9+l, ij] = sum_k rf[k,p] * ylm_s[k,l] * cg[k,ij] * nc0[ij]
(54 x 1024, computed host-side from the constant inputs, like the other
host-side weight reshapes).  Per point the device only computes
    B[pl, z] = (R[p,z] + b2[p]) * Y'[l,z]        (radial MLP + SH)
    out[z, ij] = B[:, z].T @ M                   (k=54 fp32r matmul)

Distribution: data-parallel over z across 8 NeuronCores; constants
replicated.  Full inputs in, full output out.

Precision vs the 2e-2 gate: fp32r matmuls (11-bit mantissa, 1 cyc/row),
f16 for the tiny SH/MLP operands (10-bit mantissa), and an f16 output
staged through SBUF (halves the HBM write, which is the roofline).
Expected rel-err ~1e-3, ~20x inside the gate.

Pipeline (per 512-point group): PE transposes channel planes, computes
the radial MLP + Y broadcast + eight 512-col k-matmuls; ACT/DVE drain
PSUM to f16 SBUF; one 1 MiB DMA per group writes out.  Stages are
software-pipelined two groups deep so PE never waits on ACT/DVE.
"""

import numpy as np

import concourse.bass as bass
import concourse.tile as tile
from concourse import bacc, mybir
from concourse.bass_utils import run_bass_kernel_spmd

F32 = mybir.dt.float32
F32R = mybir.dt.float32r
F16 = mybir.dt.float16
ALU = mybir.AluOpType
ACTF = mybir.ActivationFunctionType

# Problem shape (hardcoded per contract)
Z, KDIM, DO, DI, NPATH, H = 100000, 1024, 32, 32, 6, 128
IJ = DO * DI                      # 1024
NCORES = 8
ZC = Z // NCORES                  # 12500 points per core
T = 100                           # point tiles of 128 -> ZC padded to 12800
ZC_PAD = 128 * T
TB = 4                            # tiles per group
NG = T // TB                      # 25 groups of 512 points
NCH = 10                          # channels: radii, ones, 8 scaled monomials
GZ = 128 * TB                     # 512 points per group

# Real spherical harmonic constants (l=0,1,2), folded into M host-side
C0 = 0.28209479177387814
C1 = 0.4886025119029199
C2A = 1.0925484305920792
C2B = 0.31539156525252005
C2C = 0.5462742152960396
YLM_SCALE = np.array([C0, C1, C1, C1, C2A, C2A, C2B, C2A, C2C], dtype=np.float64)

_CACHE = {}


def _build_program():
    nc = bacc.Bacc("TRN2", target_bir_lowering=False, debug=False,
                   num_devices=NCORES)

    # ---- per-core DRAM I/O ----
    rpl = nc.dram_tensor("rpl", [128, 3 * T], F32, kind="ExternalInput").ap()
    m2d = nc.dram_tensor("m2d", [118, IJ], F16, kind="ExternalInput").ap()
    w1e4 = nc.dram_tensor("w1e4", [NCH * TB, TB * 128], F16, kind="ExternalInput").ap()
    ey4 = nc.dram_tensor("ey4", [NCH * TB, TB * 54], F16, kind="ExternalInput").ap()
    w2e = nc.dram_tensor("w2e", [H, 54], F16, kind="ExternalInput").ap()
    b1c = nc.dram_tensor("b1c", [H, 1], F32, kind="ExternalInput").ap()
    b2r = nc.dram_tensor("b2r", [118, 1], F32, kind="ExternalInput").ap()
    identd = nc.dram_tensor("identd", [128, 128], F16, kind="ExternalInput").ap()
    out = nc.dram_tensor("out", [ZC_PAD, IJ], F16, kind="ExternalOutput").ap()

    with tile.TileContext(nc) as tc:
        with tc.tile_pool(name="const", bufs=1) as cpool, \
             tc.tile_pool(name="tps", bufs=1, space="PSUM") as tps_pool, \
             tc.tile_pool(name="hps", bufs=1, space="PSUM") as hps_pool, \
             tc.tile_pool(name="ryps", bufs=1, space="PSUM") as ry_pool, \
             tc.tile_pool(name="kps", bufs=1, space="PSUM") as kps_pool, \
             tc.tile_pool(name="tsb", bufs=3) as tpool, \
             tc.tile_pool(name="hsb", bufs=2) as hpool, \
             tc.tile_pool(name="bsb", bufs=2) as bpool, \
             tc.tile_pool(name="rbsb", bufs=2) as rbpool, \
             tc.tile_pool(name="kout", bufs=2) as kpool:

            # ---- resident constants (rpl first so phase 2 starts early) ----
            rpl_sb = cpool.tile([128, 3 * T], F32)
            nc.sync.dma_start(rpl_sb[:], rpl[:])
            m2_sb = cpool.tile([118, IJ], F16)
            nc.sync.dma_start(m2_sb[:], m2d[:])
            w1e_sb = cpool.tile([NCH * TB, TB * 128], F16)
            nc.sync.dma_start(w1e_sb[:], w1e4[:])
            ey4_sb = cpool.tile([NCH * TB, TB * 54], F16)
            nc.sync.dma_start(ey4_sb[:], ey4[:])
            w2e_sb = cpool.tile([H, 54], F16)
            nc.sync.dma_start(w2e_sb[:], w2e[:])
            b1_sb = cpool.tile([H, 1], F32)
            nc.sync.dma_start(b1_sb[:], b1c[:])
            b2_sb = cpool.tile([118, 1], F32)
            nc.sync.dma_start(b2_sb[:], b2r[:])
            id_sb = cpool.tile([128, 128], F16)
            nc.sync.dma_start(id_sb[:], identd[:])

            # =========================================================
            # Phase 2: per-point channel planes [128, T] (f16 storage):
            # radii, ones, y/r, z/r, x/r, xy/r2, yz/r2, (3zz-r2)/r2,
            # xz/r2, (xx-yy)/r2   (channel-interleaved: col = t*NCH + c)
            # =========================================================
            chan = cpool.tile([128, NCH * T], F16)
            aux = cpool.tile([128, 8 * T], F32)

            x = rpl_sb[:, 0:T]
            y = rpl_sb[:, T:2 * T]
            z = rpl_sb[:, 2 * T:3 * T]

            def ax(i):
                return aux[:, i * T:(i + 1) * T]

            xx, yy, zz, r2, inv2, va, t3, t5 = (ax(i) for i in range(8))
            chan_v = chan[:].rearrange("p (t c) -> p c t", c=NCH)
            ch = [chan_v[:, i, :] for i in range(NCH)]

            nc.vector.tensor_tensor(xx, x, x, ALU.mult)
            nc.vector.tensor_tensor(yy, y, y, ALU.mult)
            nc.vector.tensor_tensor(zz, z, z, ALU.mult)
            nc.vector.tensor_tensor(r2, xx, yy, ALU.add)
            nc.vector.tensor_tensor(r2, r2, zz, ALU.add)
            nc.vector.reciprocal(inv2, r2)                       # 1/r2
            nc.scalar.sqrt(va, inv2)                             # 1/r (~7e-6)
            nc.vector.tensor_tensor(ch[0], r2, va, ALU.mult)     # radii
            nc.vector.memset(ch[1], 1.0)                         # l=0
            nc.vector.tensor_tensor(ch[2], y, va, ALU.mult)      # y/r
            nc.vector.tensor_tensor(ch[3], z, va, ALU.mult)      # z/r
            nc.vector.tensor_tensor(ch[4], x, va, ALU.mult)      # x/r
            nc.vector.tensor_tensor(ch[5], ch[4], ch[2], ALU.mult)  # xy/r2
            nc.vector.tensor_tensor(ch[6], ch[2], ch[3], ALU.mult)  # yz/r2
            nc.vector.scalar_tensor_tensor(t3, zz, 3.0, r2, ALU.mult,
                                           ALU.subtract)
            nc.vector.tensor_tensor(ch[7], t3, inv2, ALU.mult)   # (3zz-r2)/r2
            nc.vector.tensor_tensor(ch[8], ch[4], ch[3], ALU.mult)  # xz/r2
            nc.vector.scalar_tensor_tensor(t5, yy, -1.0, xx, ALU.mult, ALU.add)
            nc.vector.tensor_tensor(ch[9], t5, inv2, ALU.mult)   # (xx-yy)/r2

            # =========================================================
            # Phase 3: software-pipelined main loop over 25 groups
            #   stage A(g): PE transpose -> DVE copy to SBUF f16
            #   stage B(g): radial MLP (h, relu, r) + Y broadcast + B
            #   stage C(g): 8 k-matmuls + 4 PSUM->f16 drains + 1 DMA
            # =========================================================
            t_ps = tps_pool.tile([NCH * TB, 2 * 128], F16)      # 2 slots
            h_ps = hps_pool.tile([128, GZ], F32)
            ry = ry_pool.tile([118, 512], F32)                  # r|y halves
            kpsA = kps_pool.tile([128, 1024], F32)
            kpsB = kps_pool.tile([128, 1024], F32)

            # rows 54:64 of ry feed garbage lanes of the B-build; init them
            # (engine writes must start at partition 0/32/64/96)
            nc.vector.memset(ry[32:64, :], 0.0)

            t_sbs, h_sbs, b_sbs, rb_sbs, k_outs = {}, {}, {}, {}, {}

            def stage_A_pe(g):
                # transpose 4 tiles x 10 channels: [128, 40] -> [40, 128]
                nc.tensor.transpose(
                    t_ps[:, (g % 2) * 128:(g % 2) * 128 + 128],
                    chan[:, NCH * TB * g: NCH * TB * (g + 1)], id_sb[:])

            def stage_A_dve(g):
                t_sb = tpool.tile([NCH * TB, 128], F16, tag="t_sb", name="t_sb")
                t_sbs[g] = t_sb
                nc.vector.tensor_copy(
                    t_sb[:], t_ps[:, (g % 2) * 128:(g % 2) * 128 + 128])

            def stage_B_h(g):
                # hidden pre-act: h[h, z] = W1[h] * radii[z] (outer product)
                for dt in range(TB):
                    nc.tensor.matmul(
                        h_ps[:, dt * 128:(dt + 1) * 128],
                        w1e_sb[:, dt * 128:(dt + 1) * 128],
                        t_sbs[g][:], start=True, stop=True)

            def stage_B_relu(g):
                h_sb = hpool.tile([128, GZ], F16, tag="h_sb", name="h_sb")
                h_sbs[g] = h_sb
                nc.scalar.activation(h_sb[:], h_ps[:], ACTF.Relu, bias=b1_sb[:])

            def stage_B_r(g):
                # R[pl, z]: z halves at partition rows 0 and 64
                h_sb = h_sbs[g]
                nc.tensor.matmul(ry[0:54, 0:256], w2e_sb[:], h_sb[:, 0:256],
                                 start=True, stop=True)
                nc.tensor.matmul(ry[64:118, 0:256], w2e_sb[:], h_sb[:, 256:512],
                                 start=True, stop=True)

            def stage_B_y(g):
                # Y'[pl, z] broadcast: selection matmul per point tile
                t_sb = t_sbs[g]
                for dt in range(TB):
                    p0 = 0 if dt < 2 else 64
                    c0 = 256 + (dt % 2) * 128
                    nc.tensor.matmul(
                        ry[p0:p0 + 54, c0:c0 + 128],
                        ey4_sb[:, dt * 54:(dt + 1) * 54],
                        t_sb[:], start=True, stop=True)

            def stage_B_rb(g):
                # R + b2: ACT bias-add while draining PSUM -> SBUF f16
                rb_sb = rbpool.tile([118, 256], F16, tag="rb_sb", name="rb_sb")
                rb_sbs[g] = rb_sb
                nc.scalar.activation(rb_sb[:], ry[:, 0:256], ACTF.Identity,
                                     bias=b2_sb[:])

            def stage_B_b(g):
                # B = (R + b2) * Y'  (one PSUM operand, f32r SBUF out)
                b_sb = bpool.tile([118, 256], F16, tag="b_sb", name="b_sb")
                b_sbs[g] = b_sb
                nc.vector.tensor_tensor(
                    b_sb[:], rb_sbs[g][:], ry[:, 256:512], ALU.mult)

            def kmm(g, m):
                # k-matmul m = dt*2 + half -> kpsA for dt 0,2 / kpsB for 1,3
                dt, half = divmod(m, 2)
                kp = kpsA if dt % 2 == 0 else kpsB
                b_sb = b_sbs[g]
                p0 = 0 if dt < 2 else 64
                c0 = (dt % 2) * 128
                nc.tensor.matmul(
                    kp[:, half * 512:(half + 1) * 512],
                    b_sb[p0:p0 + 54, c0:c0 + 128],
                    m2_sb[p0:p0 + 54, half * 512:(half + 1) * 512],
                    start=True, stop=True)

            def stage_C_new_kout(g):
                k_outs[g] = kpool.tile([128, TB * IJ], F16, tag="k_out", name="k_out")

            def conv(g, q, eng):
                # drain quarter q: kps{A,B} [128,1024] f32 -> f16 staging
                kp = kpsA if q % 2 == 0 else kpsB
                dest = k_outs[g][:, q * 1024:(q + 1) * 1024]
                if eng == "act":
                    nc.scalar.copy(dest, kp[:])
                else:
                    nc.vector.tensor_copy(dest, kp[:])

            def stage_C_dma(g):
                og = out[GZ * g:GZ * (g + 1), :].rearrange(
                    "(dt pg) ij -> pg dt ij", dt=TB)
                nc.sync.dma_start(
                    og, k_outs[g][:].rearrange("pg (dt ij) -> pg dt ij", dt=TB))

            # pipelined emission: gA = i, gB = i-1, gC = i-2
            for i in range(NG + 2):
                gA, gB, gC = i, i - 1, i - 2
                vA, vB, vC = gA < NG, 0 <= gB < NG, 0 <= gC

                if vC:
                    stage_C_new_kout(gC)
                    kmm(gC, 0)
                    kmm(gC, 1)
                if vA:
                    stage_A_pe(gA)
                    stage_A_dve(gA)
                if vB:
                    stage_B_h(gB)
                    stage_B_relu(gB)
                if vC:
                    conv(gC, 0, "act")     # after kmm 0,1 (kpsA)
                    kmm(gC, 2)
                    kmm(gC, 3)
                    conv(gC, 1, "dve")     # after kmm 2,3 (kpsB)
                if vB:
                    stage_B_r(gB)
                    stage_B_y(gB)
                    stage_B_rb(gB)
                    stage_B_b(gB)
                if vC:
                    kmm(gC, 4)
                    kmm(gC, 5)
                    conv(gC, 2, "act")     # kpsA round 2
                    kmm(gC, 6)
                    kmm(gC, 7)
                    conv(gC, 3, "dve")     # kpsB round 2
                    stage_C_dma(gC)

    nc.compile()
    return nc


def _get_program():
    if "nc" not in _CACHE:
        _CACHE["nc"] = _build_program()
    return _CACHE["nc"]


def _host_prep(r, W1, b1, W2, b2, cg, ylm_mix, rf_mix, norm_coef):
    r = np.asarray(r, dtype=np.float32)
    W1 = np.asarray(W1, dtype=np.float32)
    b1 = np.asarray(b1, dtype=np.float32)
    W2 = np.asarray(W2, dtype=np.float32)
    b2 = np.asarray(b2, dtype=np.float32)
    cg = np.asarray(cg, dtype=np.float32)
    ylm_mix = np.asarray(ylm_mix, dtype=np.float32)
    rf_mix = np.asarray(rf_mix, dtype=np.float32)
    norm_coef = np.asarray(norm_coef, dtype=np.float32)

    # constant fold: M[p*9+l, ij] = sum_k rf[k,p] ylm_s[k,l] cg[k,ij] * nc0
    ylm_s = ylm_mix.astype(np.float64) * YLM_SCALE[None, :]
    wkpl = (rf_mix.astype(np.float64)[:, :, None] * ylm_s[:, None, :])
    m54 = wkpl.reshape(KDIM, 54).T @ cg.astype(np.float64).reshape(KDIM, IJ)
    m54 *= norm_coef[:, :, 0].astype(np.float64).reshape(1, IJ)
    m2 = np.zeros((118, IJ), dtype=np.float16)
    m2[0:54] = m54.astype(np.float16)
    m2[64:118] = m2[0:54]

    w1e4 = np.zeros((NCH * TB, TB * 128), dtype=np.float16)
    ey4 = np.zeros((NCH * TB, TB * 54), dtype=np.float16)
    for dt in range(TB):
        w1e4[NCH * dt, dt * 128:(dt + 1) * 128] = W1[0].astype(np.float16)
        for l in range(9):
            for p in range(NPATH):
                ey4[NCH * dt + 1 + l, dt * 54 + p * 9 + l] = 1.0

    b2r = np.zeros((118, 1), dtype=np.float32)
    b2r[0:54, 0] = np.repeat(b2, 9)
    b2r[64:118, 0] = b2r[0:54, 0]

    shared = {
        "m2d": m2,
        "w1e4": w1e4,
        "ey4": ey4,
        "w2e": np.ascontiguousarray(np.repeat(W2, 9, axis=1)).astype(np.float16),
        "b1c": np.ascontiguousarray(b1.reshape(H, 1)),
        "b2r": b2r,
        "identd": np.eye(128, dtype=np.float16),
    }

    in_maps = []
    for c in range(NCORES):
        rs = r[c * ZC:(c + 1) * ZC]
        rp = np.empty((ZC_PAD, 3), dtype=np.float32)
        rp[:ZC] = rs
        rp[ZC:] = np.array([1.0, 0.0, 0.0], dtype=np.float32)
        rpl = rp.reshape(T, 128, 3).transpose(1, 2, 0).reshape(128, 3 * T)
        m = dict(shared)
        m["rpl"] = np.ascontiguousarray(rpl)
        in_maps.append(m)
    return in_maps


def _run_device(in_maps, trace=False, **kw):
    nc = _get_program()
    return run_bass_kernel_spmd(nc, in_maps, core_ids=list(range(NCORES)),
                                trace=trace, **kw)


def kernel(r, W1, b1, W2, b2, cg, ylm_mix, rf_mix, norm_coef):
    r = np.asarray(r, dtype=np.float32)
    norm_coef_f = np.asarray(norm_coef, dtype=np.float32)
    in_maps = _host_prep(r, W1, b1, W2, b2, cg, ylm_mix, rf_mix, norm_coef_f)
    res = _run_device(in_maps)
    out = np.concatenate(
        [np.asarray(res.results[c]["out"])[:ZC] for c in range(NCORES)],
        axis=0).astype(np.float32)

    # points with exactly zero radius: recompute those rows exactly
    # (they use norm_coef[..., 1] and the safe-guarded Y)
    x, y, z = r[:, 0], r[:, 1], r[:, 2]
    r2 = (x * x + y * y) + z * z
    zero = r2 == np.float32(0.0)
    if np.any(zero):
        W1f = np.asarray(W1, np.float64)
        b1f = np.asarray(b1, np.float64)
        W2f = np.asarray(W2, np.float64)
        b2f = np.asarray(b2, np.float64)
        cgf = np.asarray(cg, np.float64)
        ylm = np.asarray(ylm_mix, np.float64)
        rf = np.asarray(rf_mix, np.float64)
        yzero = np.zeros(9); yzero[0] = C0
        hrow = np.maximum(0.0 * W1f[0] + b1f, 0.0)       # radii = 0
        rrow = hrow @ W2f + b2f
        rk = rf @ rrow                                    # [K]
        yk = ylm @ yzero                                  # [K]
        krow = np.einsum('k,kij->ij', rk * yk, cgf)
        krow = krow * np.asarray(norm_coef_f[:, :, 1], np.float64)
        out[zero] = krow.reshape(1, IJ).astype(np.float32)

    return out.reshape(Z, DO, DI)
